# revision 37
# baseline (speedup 1.0000x reference)
"""NeuronPool (moe_routing) Trainium2 kernel.

Expert-parallel over 8 NeuronCores: core c computes neurons [8c, 8c+8) for the
full batch, host concatenates along the neuron axis.

The fp32 baseline (~146-171us) was DMA-bound (50.3MB of weights per core at
~400GB/s).  This version:
  - streams weights as bf16 (25.2MB per core, ~63us floor): per-neuron slabs
    [128, 12288] (W1|W2|W3 packed per partition) as two SWDGE DMAs each (W1,
    then W2+W3; the last neuron splits W3 off so the final arriving bytes
    feed the shortest compute chain).
  - runs all GEMMs bf16 x bf16 into f32 PSUM (weights-moving dataflow;
    stationary xT/hT cast to bf16 by the DVE transpose-copies).
  - specializes on the actual parameter values (b1=b2=b3=0, gamma=1, beta=0
    in this model): bias/gamma/beta selector matmuls vanish and the LN
    epilogue is yo = yc * (rsqrt(var+eps) * mod_n).  A general program is
    built instead if the inputs are not of this form.
  - software-pipelines GEMM1 of neuron n+1 in 6-matmul groups between neuron
    n's gelu/transpose/GEMM2/GEMM3 phases.  The PE's HAM clock monitor does
    not count transpose-mode as busy, so without this the clock gate drops to
    1.2GHz during every transpose phase and each GEMM1 restarts cold (the
    profile showed a 6.8us-warm/3.4us-cold oscillation all run long).
  - warms the PE with real (non-transpose) matmuls while the first W1 slab
    streams in.
  - keeps the ACT engine on a single activation-table set (Gelu+Square);
    LN's 1/sqrt runs on DVE as a Quake-style rsqrt (bitcast + shift + two
    Newton steps), and the row-sum runs on DVE tensor_reduce.
Measured rel err ~4e-3 (bf16 rounding), inside the 2e-2 gate.
"""
import math
import numpy as np
from contextlib import ExitStack

import ml_dtypes
import concourse.bass as bass
import concourse.tile as tile
from concourse import bacc, mybir
from concourse.bass_utils import run_bass_kernel_spmd

N_CORES = 8
B = 32          # batch
D = 256         # model dim
HIST = 8
HID = 512
N_NEURONS = 64
NPC = N_NEURONS // N_CORES  # 8 neurons per core
IN_DIM = D * (1 + HIST)     # 2304
KC1 = IN_DIM // 128         # 18 contraction chunks for GEMM1
KC2 = HID // 128            # 4 chunks for GEMM2/GEMM3
LN_EPS = 1e-5
FMIN, FMAX = 0.5, 40.0
TICK_INTERVAL = 0.1

f32 = mybir.dt.float32
f32r = mybir.dt.float32r
bf16 = mybir.dt.bfloat16
i32 = mybir.dt.int32

# general-path packed bias row layout
B1_OFF = 0
B2_OFF = B1_OFF + HID
B3_OFF = B2_OFF + HID
BVB_LEN = B3_OFF + D        # 1280
BVG_LEN = 2 * D             # 512 (gm | bm)

# weight slab layout (bf16 elements per partition)
W1_OFF = 0                  # 18 chunks x 512
W2_OFF = KC1 * HID          # 9216, 4 chunks x 512
W3_OFF = W2_OFF + KC2 * HID  # 11264, 4 chunks x 256
SLAB_LEN = W3_OFF + KC2 * D  # 12288

QMAGIC = 0x5F3759DF
N_WARMUP = 10               # PE warm-up matmuls during the first W1 DMA

_CACHE = {}


def _build_program(fast):
    nc = bacc.Bacc("TRN2", target_bir_lowering=False, debug=False,
                   num_devices=N_CORES)

    emb = nc.dram_tensor("emb", [B, D], f32, kind="ExternalInput").ap()
    wpd = nc.dram_tensor("wpd", [128, 2, D], bf16, kind="ExternalInput").ap()
    bpd = nc.dram_tensor("bpd", [128, 2], f32, kind="ExternalInput").ap()
    histd = nc.dram_tensor("histd", [16, 128], f32, kind="ExternalInput").ap()
    eyed = nc.dram_tensor("eyed", [32, 32], f32, kind="ExternalInput").ap()
    wsd = nc.dram_tensor("wsd", [NPC, 128, SLAB_LEN], bf16, kind="ExternalInput").ap()
    if fast:
        modd = nc.dram_tensor("modd", [B, NPC], f32, kind="ExternalInput").ap()
    else:
        bvbd = nc.dram_tensor("bvbd", [NPC, BVB_LEN], bf16, kind="ExternalInput").ap()
        bvgd = nc.dram_tensor("bvgd", [NPC, BVG_LEN], f32, kind="ExternalInput").ap()
        sel8d = nc.dram_tensor("sel8d", [NPC, NPC * B], f32, kind="ExternalInput").ap()
    out = nc.dram_tensor("out", [B, NPC, D], f32, kind="ExternalOutput").ap()

    GELU = mybir.ActivationFunctionType.Gelu
    SQUARE = mybir.ActivationFunctionType.Square
    MULT = mybir.AluOpType.mult
    ADD = mybir.AluOpType.add
    ASR = mybir.AluOpType.arith_shift_right

    with tile.TileContext(nc) as tc, ExitStack() as ctx:
        # SBUF pools
        cst = ctx.enter_context(tc.tile_pool(name="cst", bufs=1))
        xtp = ctx.enter_context(tc.tile_pool(name="xtp", bufs=KC1))
        slabp = ctx.enter_context(tc.tile_pool(name="slabp", bufs=5))
        htp = ctx.enter_context(tc.tile_pool(name="htp", bufs=16))
        hp = ctx.enter_context(tc.tile_pool(name="hp", bufs=4))
        ysp = ctx.enter_context(tc.tile_pool(name="ysp", bufs=NPC))
        yp = ctx.enter_context(tc.tile_pool(name="yp", bufs=10))
        stp = ctx.enter_context(tc.tile_pool(name="stp", bufs=36))
        # PSUM pools (8 banks)
        accp = ctx.enter_context(
            tc.tile_pool(name="accp", bufs=4 if fast else 3, space="PSUM"))
        trp = ctx.enter_context(tc.tile_pool(name="trp", bufs=3, space="PSUM"))
        if not fast:
            gbp = ctx.enter_context(tc.tile_pool(name="gbp", bufs=2, space="PSUM"))

        # ---- weight slab DMA plumbing (SWDGE queue is weights-only; small
        # inputs ride the HWDGE sync queue so slab descriptors are generated
        # from t=0) ----
        def dma_slab_part(n, t, off, end, eng=None):
            (eng or nc.gpsimd).dma_start(out=t[:, off:end], in_=wsd[n][:, off:end])

        def dma_slab(n, split3=False, splitw1=False):
            # W1 parts stream on SWDGE; W2/W3 parts stream on HWDGE (sync) in
            # parallel -- the two DGE paths share the 16 SDMA engines, and
            # driving both gets the combined stream closer to the fabric rate.
            # Neuron 0's W1 goes via HWDGE in thirds: it starts at ~0.6us
            # while the SWDGE path spends ~5us booting the Q7.
            t = slabp.tile([128, SLAB_LEN], bf16, tag="slab", name="slab")
            if splitw1:
                third = KC1 // 3 * HID
                dma_slab_part(n, t, W1_OFF, W1_OFF + third, eng=nc.sync)
                dma_slab_part(n, t, W1_OFF + third, W1_OFF + 2 * third, eng=nc.sync)
                dma_slab_part(n, t, W1_OFF + 2 * third, W2_OFF, eng=nc.sync)
            else:
                dma_slab_part(n, t, W1_OFF, W2_OFF)
            if split3:
                dma_slab_part(n, t, W2_OFF, W3_OFF, eng=nc.sync)
                dma_slab_part(n, t, W3_OFF, SLAB_LEN, eng=nc.sync)
            else:
                dma_slab_part(n, t, W2_OFF, SLAB_LEN, eng=nc.sync)
            return t

        # ---- constants (first on the SWDGE queue: ~140KB, delays the slab
        # stream by <0.5us but lets x-setup + PE warm-up start at ~1.5us) ----
        eye = cst.tile([32, 32], f32, tag="eye")
        nc.gpsimd.dma_start(out=eye[:], in_=eyed)
        onesb = cst.tile([128, 32], bf16, tag="onesb")
        nc.vector.memset(onesb[:], 1.0)
        magic = cst.tile([B, 1], i32, tag="magic")
        nc.vector.memset(magic[:], QMAGIC)
        bpt = cst.tile([128, 2], f32, tag="bpt")
        nc.gpsimd.dma_start(out=bpt[:], in_=bpd)
        if fast:
            modt = cst.tile([B, NPC], f32, tag="modt")
            nc.gpsimd.dma_start(out=modt[:], in_=modd)
        else:
            bvb = cst.tile([NPC, BVB_LEN], bf16, tag="bvb")
            nc.gpsimd.dma_start(out=bvb[:], in_=bvbd)
            bvg = cst.tile([NPC, BVG_LEN], f32r, tag="bvg")
            nc.gpsimd.dma_start(out=bvg[:], in_=bvgd)
            sel8r = cst.tile([NPC, NPC * B], f32r, tag="sel8r")
            nc.gpsimd.dma_start(out=sel8r[:], in_=sel8d)
            sel8b = cst.tile([NPC, NPC * B], bf16, tag="sel8b")
            nc.vector.tensor_copy(sel8b[:], sel8r[:])

        def w1c(t, k):
            return t[:, W1_OFF + k * HID:W1_OFF + (k + 1) * HID]

        def w2c(t, j):
            return t[:, W2_OFF + j * HID:W2_OFF + (j + 1) * HID]

        def w3c(t, j):
            return t[:, W3_OFF + j * D:W3_OFF + (j + 1) * D]

        # ---- x setup: xT chunks [128, 32] bf16, k = 0..17 ----
        xT = []
        xe = cst.tile([B, D], f32, tag="xe")
        nc.gpsimd.dma_start(out=xe[:], in_=emb)
        wpt = cst.tile([128, 2, D], bf16, tag="wpt")
        nc.gpsimd.dma_start(out=wpt[:], in_=wpd)
        xeT = []
        for k in range(2):
            pt = trp.tile([128, 32], f32, tag="tr")
            nc.tensor.transpose(pt[:], xe[:, k * 128:(k + 1) * 128], eye[:])
            st = cst.tile([128, 32], bf16, tag=f"xeT{k}")
            nc.vector.tensor_copy(st[:], pt[:])
            xeT.append(st)
        for m in range(2):
            pp = trp.tile([128, 32], f32, tag="tr")
            for k in range(2):
                nc.tensor.matmul(pp[:], wpt[:, k, m * 128:(m + 1) * 128], xeT[k][:],
                                 start=(k == 0), stop=(k == 1))
            xt = xtp.tile([128, 32], bf16, tag="xt")
            nc.vector.tensor_scalar_add(xt[:], pp[:], bpt[:, m:m + 1])
            xT.append(xt)

        ht = cst.tile([16, 128], f32, tag="ht")
        nc.gpsimd.dma_start(out=ht[:], in_=histd)

        # weight slabs follow the small inputs on the SWDGE queue
        slabs = {0: dma_slab(0, splitw1=True)}
        if fast:
            slabs[1] = dma_slab(1)
        pt = trp.tile([128, 16], f32, tag="tr")
        nc.tensor.transpose(pt[:], ht[:], eye[0:16, 0:16])
        histT = cst.tile([128, 16], f32, tag="histT")
        nc.vector.tensor_copy(histT[:], pt[:])
        for c in range(16):
            xt = xtp.tile([128, 32], bf16, tag="xt")
            nc.vector.tensor_scalar_mul(xt[:], onesb[:], histT[:, c:c + 1])
            xT.append(xt)

        # PE warm-up with real matmuls (transpose-mode does NOT count as
        # PE-busy for the HAM clock monitor): open the clock gate to 2.4GHz
        # while the first W1 third streams in.  Ping-pong two PSUM tiles so
        # the drains overlap.
        wps0 = trp.tile([B, D], f32, tag="tr")
        wps1 = trp.tile([B, D], f32, tag="tr")
        for w in range(N_WARMUP):
            nc.tensor.matmul(wps0[:] if w % 2 == 0 else wps1[:],
                             xeT[0][:], wpt[:, 0, :], start=True, stop=True)

        # ---- pipeline pieces ----
        ycs = {}
        stats = {}

        def g1_alloc():
            return accp.tile([B, HID], f32, tag="acc", name="p1")

        def g1_group(n, t, p1, k0, k1):
            for k in range(k0, k1):
                nc.tensor.matmul(p1[:], xT[k][:], w1c(t, k),
                                 start=(fast and k == 0), stop=(k == KC1 - 1))

        def g1_bias(n, p1):
            nc.tensor.matmul(p1[:], sel8b[:, n * B:(n + 1) * B],
                             bvb[:, B1_OFF:B1_OFF + HID], start=True, stop=False)

        def gelu_chunks(p, h):
            for j in range(KC2):
                nc.scalar.activation(h[:, j * 128:(j + 1) * 128],
                                     p[:, j * 128:(j + 1) * 128], GELU)

        def transpose4(h):
            hT = []
            for j in range(KC2):
                pt = trp.tile([128, 32], f32, tag="tr")
                nc.tensor.transpose(pt[:], h[:, j * 128:(j + 1) * 128], eye[:])
                st = htp.tile([128, 32], bf16, tag="hT")
                nc.vector.tensor_copy(st[:], pt[:])
                hT.append(st)
            return hT

        def warm_mms(k):
            # keep the HAM clock gate open through phases with no real
            # matmul work (the last neuron's epilogue)
            for w in range(k):
                nc.tensor.matmul(wps0[:] if w % 2 == 0 else wps1[:],
                                 xeT[0][:], wpt[:, 0, :], start=True, stop=True)

        def gemm2_mms(n, t, h1T):
            p2 = accp.tile([B, HID], f32, tag="acc")
            if not fast:
                nc.tensor.matmul(p2[:], sel8b[:, n * B:(n + 1) * B],
                                 bvb[:, B2_OFF:B2_OFF + HID], start=True, stop=False)
            for j in range(KC2):
                nc.tensor.matmul(p2[:], h1T[j][:], w2c(t, j),
                                 start=(fast and j == 0), stop=(j == KC2 - 1))
            return p2

        def w2b(t, k, m):
            # lhsT chunk for weights-stationary GEMM2: [hid1 128k..+128 (K),
            # hid2 128m..+128 (M)] — slab layout already matches
            off = W2_OFF + k * HID + m * 128
            return t[:, off:off + 128]

        def gemm2_b(n, t, h1T):
            # weights-stationary GEMM2: psum2T [128, 4, 32] comes out with
            # hid2 on partitions, so gelu2 directly produces the transposed
            # h2T chunks GEMM3 needs (no layer-2 PE transposes / DVE casts)
            p2t = accp.tile([128, KC2, B], f32, tag="acc", name="p2t")
            for m in range(KC2):
                for k in range(KC2):
                    nc.tensor.matmul(p2t[:, m, :], w2b(t, k, m), h1T[k][:],
                                     start=(k == 0), stop=(k == KC2 - 1))
            h2T = []
            for m in range(KC2):
                st = htp.tile([128, 32], bf16, tag="hT", name="h2T")
                nc.scalar.activation(st[:], p2t[:, m, :], GELU)
                h2T.append(st)
            return h2T

        def gemm3_stats(n, t, h2T):
            p3 = accp.tile([B, D], f32, tag="acc")
            if not fast:
                nc.tensor.matmul(p3[:], sel8b[:, n * B:(n + 1) * B],
                                 bvb[:, B3_OFF:B3_OFF + D], start=True, stop=False)
            for j in range(KC2):
                nc.tensor.matmul(p3[:], h2T[j][:], w3c(t, j),
                                 start=(fast and j == 0), stop=(j == KC2 - 1))
            # LN stats: rs = sum(y) on DVE; yc = y - rs/D; ssq = sum(yc^2)
            # on ACT (Square shares Gelu's table set: no table reload)
            rs = stp.tile([B, 1], f32, tag="st")
            nc.vector.tensor_reduce(rs[:], p3[:], mybir.AxisListType.X, ADD)
            nmu = stp.tile([B, 1], f32, tag="st")
            nc.vector.tensor_scalar_mul(nmu[:], rs[:], -1.0 / D)
            yc = ysp.tile([B, D], f32, tag="ys")
            nc.vector.tensor_scalar_add(yc[:], p3[:], nmu[:])
            sqs = yp.tile([B, D], f32, tag="y")
            ssq = stp.tile([B, 1], f32, tag="st")
            nc.scalar.activation(sqs[:], yc[:], SQUARE, accum_out=ssq[:])
            ycs[n] = yc
            stats[n] = ssq

        def rsqrt_dve(ssq):
            # inv = 1/sqrt(ssq/D + eps): Quake seed + two Newton steps (DVE)
            msq = stp.tile([B, 1], f32, tag="st")
            nc.vector.tensor_scalar(msq[:], ssq[:], 1.0 / D, LN_EPS, MULT, ADD)
            shi = stp.tile([B, 1], i32, tag="sti")
            nc.vector.tensor_scalar(shi[:], msq[:].bitcast(i32), 1, None, ASR)
            x0i = stp.tile([B, 1], i32, tag="sti")
            nc.vector.tensor_sub(x0i[:], magic[:], shi[:])
            x0 = x0i[:].bitcast(f32)
            u = stp.tile([B, 1], f32, tag="st")
            nc.vector.scalar_tensor_tensor(u[:], x0, x0, msq[:], MULT, MULT)
            v = stp.tile([B, 1], f32, tag="st")
            nc.vector.tensor_scalar(v[:], u[:], -0.5, 1.5, MULT, ADD)
            x1 = stp.tile([B, 1], f32, tag="st")
            nc.vector.tensor_mul(x1[:], v[:], x0)
            u2 = stp.tile([B, 1], f32, tag="st")
            nc.vector.scalar_tensor_tensor(u2[:], x1[:], x1[:], msq[:], MULT, MULT)
            v2 = stp.tile([B, 1], f32, tag="st")
            nc.vector.tensor_scalar(v2[:], u2[:], -0.5, 1.5, MULT, ADD)
            inv = stp.tile([B, 1], f32, tag="st")
            nc.vector.tensor_mul(inv[:], v2[:], x1[:])
            return inv

        def emit_B(n):
            yc, ssq = ycs[n], stats[n]
            inv = rsqrt_dve(ssq)
            if fast:
                invm = stp.tile([B, 1], f32, tag="st")
                nc.vector.tensor_mul(invm[:], inv[:], modt[:, n:n + 1])
                yo = yp.tile([B, D], f32, tag="y")
                nc.vector.tensor_scalar_mul(yo[:], yc[:], invm[:])
            else:
                gb = gbp.tile([B, 2 * D], f32, tag="gb")
                nc.tensor.matmul(gb[:], sel8r[:, n * B:(n + 1) * B], bvg[:],
                                 start=True, stop=True)
                yg = yp.tile([B, D], f32, tag="y")
                nc.vector.scalar_tensor_tensor(
                    yg[:], yc[:], inv[:], gb[:, 0:D], MULT, MULT)
                yo = yp.tile([B, D], f32, tag="y")
                nc.vector.tensor_add(yo[:], yg[:], gb[:, D:2 * D])
            nc.sync.dma_start(out=out[:, n, :], in_=yo[:])

        if fast:
            # software-pipelined: GEMM1 of neuron n+1 interleaved (in 6-MM
            # groups) with neuron n's gelu/transpose/GEMM2/GEMM3 phases so
            # the PE always has real matmul work in every HAM window.
            p1s = {0: g1_alloc()}
            g1_group(0, slabs[0], p1s[0], 0, KC1)
            for n in range(NPC):
                nxt = n + 1
                if nxt < NPC:
                    if nxt not in slabs:
                        slabs[nxt] = dma_slab(nxt, split3=(nxt == NPC - 1))
                    p1s[nxt] = g1_alloc()
                h1 = hp.tile([B, HID], f32, tag="h")
                gelu_chunks(p1s[n], h1)
                if nxt < NPC:
                    g1_group(nxt, slabs[nxt], p1s[nxt], 0, 6)
                else:
                    warm_mms(3)
                h1T = transpose4(h1)
                if nxt < NPC:
                    g1_group(nxt, slabs[nxt], p1s[nxt], 6, 12)
                else:
                    warm_mms(3)
                h2T = gemm2_b(n, slabs[n], h1T)
                if nxt < NPC:
                    g1_group(nxt, slabs[nxt], p1s[nxt], 12, KC1)
                else:
                    warm_mms(3)
                gemm3_stats(n, slabs[n], h2T)
                if n > 0:
                    emit_B(n - 1)
            emit_B(NPC - 1)
        else:
            # general path: plain per-neuron pipeline with B lagging one
            def emit_A(n, split3=False):
                t = slabs[n] if n in slabs else dma_slab(n, split3=split3)
                p1 = g1_alloc()
                g1_bias(n, p1)
                g1_group(n, t, p1, 0, KC1)
                h1 = hp.tile([B, HID], f32, tag="h")
                gelu_chunks(p1, h1)
                h1T = transpose4(h1)
                p2 = gemm2_mms(n, t, h1T)
                h2 = hp.tile([B, HID], f32, tag="h")
                gelu_chunks(p2, h2)
                h2T = transpose4(h2)
                gemm3_stats(n, t, h2T)

            for n in range(NPC):
                emit_A(n, split3=(n == NPC - 1))
                if n > 0:
                    emit_B(n - 1)
            emit_B(NPC - 1)

    nc.compile()
    return nc


def _get_program(fast):
    key = "fast" if fast else "general"
    if key not in _CACHE:
        _CACHE[key] = _build_program(fast)
    return _CACHE[key]


def _prep_in_maps(input_embedding, pre_activations, Wp, bp, W1, b1, W2, b2, W3,
                  b3, gamma, beta, tick):
    emb = np.asarray(input_embedding, dtype=np.float32)
    hist = np.asarray(pre_activations, dtype=np.float32)
    Wp = np.asarray(Wp, dtype=np.float32)
    bp = np.asarray(bp, dtype=np.float32)
    W1 = np.asarray(W1, dtype=np.float32)
    b1 = np.asarray(b1, dtype=np.float32)
    W2 = np.asarray(W2, dtype=np.float32)
    b2 = np.asarray(b2, dtype=np.float32)
    W3 = np.asarray(W3, dtype=np.float32)
    b3 = np.asarray(b3, dtype=np.float32)
    gamma = np.asarray(gamma, dtype=np.float32)
    beta = np.asarray(beta, dtype=np.float32)

    fast = (not b1.any() and not b2.any() and not b3.any() and not beta.any()
            and bool(np.all(gamma == 1.0)))

    i = np.arange(N_NEURONS, dtype=np.float64)
    freq = FMIN * (FMAX / FMIN) ** (i / (N_NEURONS - 1))
    phase = np.mod(i * 2.3571, 2.0 * math.pi)
    t = float(np.asarray(tick)) * TICK_INTERVAL
    mod = (1.0 + 0.5 * np.sin(2.0 * math.pi * freq * t + phase)).astype(np.float32)

    histd = np.ascontiguousarray(hist.reshape(16, 128))
    bpd = np.ascontiguousarray(bp.reshape(2, 128).T)
    eyed = np.eye(32, dtype=np.float32)
    wpd = np.ascontiguousarray(
        Wp.reshape(2, 128, D).transpose(1, 0, 2)).astype(ml_dtypes.bfloat16)

    # weight slab: per (neuron, partition) one contiguous bf16 run
    # [W1 18x512 | W2 4x512 | W3 4x256], partition = contraction row % 128
    W1r = W1.reshape(N_NEURONS, KC1, 128, HID).transpose(0, 2, 1, 3) \
        .reshape(N_NEURONS, 128, KC1 * HID)
    W2r = W2.reshape(N_NEURONS, KC2, 128, HID).transpose(0, 2, 1, 3) \
        .reshape(N_NEURONS, 128, KC2 * HID)
    W3r = W3.reshape(N_NEURONS, KC2, 128, D).transpose(0, 2, 1, 3) \
        .reshape(N_NEURONS, 128, KC2 * D)
    wslab = np.ascontiguousarray(
        np.concatenate([W1r, W2r, W3r], axis=2)).astype(ml_dtypes.bfloat16)

    in_maps = []
    for c in range(N_CORES):
        s = slice(c * NPC, (c + 1) * NPC)
        im = {
            "emb": emb,
            "wpd": wpd,
            "bpd": bpd,
            "histd": histd,
            "eyed": eyed,
            "wsd": wslab[s],
        }
        if fast:
            im["modd"] = np.ascontiguousarray(
                np.tile(mod[s][None, :], (B, 1)).astype(np.float32))
        else:
            gm = (gamma * mod[:, None]).astype(np.float32)
            bm = (beta * mod[:, None]).astype(np.float32)
            sel8 = np.zeros((NPC, NPC * B), dtype=np.float32)
            for n in range(NPC):
                sel8[n, n * B:(n + 1) * B] = 1.0
            im["bvbd"] = np.ascontiguousarray(
                np.concatenate([b1[s], b2[s], b3[s]], axis=1)
            ).astype(ml_dtypes.bfloat16)
            im["bvgd"] = np.ascontiguousarray(
                np.concatenate([gm[s], bm[s]], axis=1))
            im["sel8d"] = sel8
        in_maps.append(im)
    return fast, in_maps


def run(inputs, trace=False):
    fast, in_maps = _prep_in_maps(**inputs)
    nc = _get_program(fast)
    br = run_bass_kernel_spmd(nc, in_maps, core_ids=list(range(N_CORES)),
                              trace=trace)
    out = np.concatenate([r["out"] for r in br.results], axis=1)
    return np.ascontiguousarray(out, dtype=np.float32), br


def kernel(**inputs) -> np.ndarray:
    out, _ = run(inputs, trace=False)
    return out


# revision 38
# speedup vs baseline: 1.1702x; 1.1702x over previous
"""NeuronPool (moe_routing) Trainium2 kernel.

Expert-parallel over 8 NeuronCores: core c computes neurons [8c, 8c+8) for the
full batch, host concatenates along the neuron axis.

The fp32 baseline (~146-171us) was DMA-bound (50.3MB of weights per core at
~400GB/s).  This version:
  - streams weights as bf16 (25.2MB per core, ~63us floor): per-neuron slabs
    [128, 12288] (W1|W2|W3 packed per partition) as two SWDGE DMAs each (W1,
    then W2+W3; the last neuron splits W3 off so the final arriving bytes
    feed the shortest compute chain).
  - runs all GEMMs bf16 x bf16 into f32 PSUM (weights-moving dataflow;
    stationary xT/hT cast to bf16 by the DVE transpose-copies).
  - specializes on the actual parameter values (b1=b2=b3=0, gamma=1, beta=0
    in this model): bias/gamma/beta selector matmuls vanish and the LN
    epilogue is yo = yc * (rsqrt(var+eps) * mod_n).  A general program is
    built instead if the inputs are not of this form.
  - software-pipelines GEMM1 of neuron n+1 in 6-matmul groups between neuron
    n's gelu/transpose/GEMM2/GEMM3 phases.  The PE's HAM clock monitor does
    not count transpose-mode as busy, so without this the clock gate drops to
    1.2GHz during every transpose phase and each GEMM1 restarts cold (the
    profile showed a 6.8us-warm/3.4us-cold oscillation all run long).
  - warms the PE with real (non-transpose) matmuls while the first W1 slab
    streams in.
  - keeps the ACT engine on a single activation-table set (Gelu+Square);
    LN's 1/sqrt runs on DVE as a Quake-style rsqrt (bitcast + shift + two
    Newton steps), and the row-sum runs on DVE tensor_reduce.
Measured rel err ~4e-3 (bf16 rounding), inside the 2e-2 gate.
"""
import math
import numpy as np
from contextlib import ExitStack

import ml_dtypes
import concourse.bass as bass
import concourse.tile as tile
from concourse import bacc, mybir
from concourse.bass_utils import run_bass_kernel_spmd

N_CORES = 8
B = 32          # batch
D = 256         # model dim
HIST = 8
HID = 512
N_NEURONS = 64
NPC = N_NEURONS // N_CORES  # 8 neurons per core
IN_DIM = D * (1 + HIST)     # 2304
KC1 = IN_DIM // 128         # 18 contraction chunks for GEMM1
KC2 = HID // 128            # 4 chunks for GEMM2/GEMM3
LN_EPS = 1e-5
FMIN, FMAX = 0.5, 40.0
TICK_INTERVAL = 0.1

f32 = mybir.dt.float32
f32r = mybir.dt.float32r
bf16 = mybir.dt.bfloat16
i32 = mybir.dt.int32

# general-path packed bias row layout
B1_OFF = 0
B2_OFF = B1_OFF + HID
B3_OFF = B2_OFF + HID
BVB_LEN = B3_OFF + D        # 1280
BVG_LEN = 2 * D             # 512 (gm | bm)

# weight slab layout (bf16 elements per partition)
W1_OFF = 0                  # 18 chunks x 512
W2_OFF = KC1 * HID          # 9216, 4 chunks x 512
W3_OFF = W2_OFF + KC2 * HID  # 11264, 4 chunks x 256
SLAB_LEN = W3_OFF + KC2 * D  # 12288

QMAGIC = 0x5F3759DF
N_WARMUP = 10               # PE warm-up matmuls during the first W1 DMA

_CACHE = {}


def _build_program(fast):
    nc = bacc.Bacc("TRN2", target_bir_lowering=False, debug=False,
                   num_devices=N_CORES)

    emb = nc.dram_tensor("emb", [B, D], f32, kind="ExternalInput").ap()
    wpd = nc.dram_tensor("wpd", [128, 2, D], bf16, kind="ExternalInput").ap()
    bpd = nc.dram_tensor("bpd", [128, 2], f32, kind="ExternalInput").ap()
    histd = nc.dram_tensor("histd", [16, 128], f32, kind="ExternalInput").ap()
    eyed = nc.dram_tensor("eyed", [32, 32], f32, kind="ExternalInput").ap()
    wsd = nc.dram_tensor("wsd", [NPC, 128, SLAB_LEN], bf16, kind="ExternalInput").ap()
    if fast:
        modd = nc.dram_tensor("modd", [B, NPC], f32, kind="ExternalInput").ap()
    else:
        bvbd = nc.dram_tensor("bvbd", [NPC, BVB_LEN], bf16, kind="ExternalInput").ap()
        bvgd = nc.dram_tensor("bvgd", [NPC, BVG_LEN], f32, kind="ExternalInput").ap()
        sel8d = nc.dram_tensor("sel8d", [NPC, NPC * B], f32, kind="ExternalInput").ap()
    out = nc.dram_tensor("out", [B, NPC, D], f32, kind="ExternalOutput").ap()

    GELU = mybir.ActivationFunctionType.Gelu
    SQUARE = mybir.ActivationFunctionType.Square
    MULT = mybir.AluOpType.mult
    ADD = mybir.AluOpType.add
    ASR = mybir.AluOpType.arith_shift_right

    with tile.TileContext(nc) as tc, ExitStack() as ctx:
        # SBUF pools
        cst = ctx.enter_context(tc.tile_pool(name="cst", bufs=1))
        xtp = ctx.enter_context(tc.tile_pool(name="xtp", bufs=KC1))
        slabp = ctx.enter_context(tc.tile_pool(name="slabp", bufs=5))
        htp = ctx.enter_context(tc.tile_pool(name="htp", bufs=16))
        hp = ctx.enter_context(tc.tile_pool(name="hp", bufs=4))
        ysp = ctx.enter_context(tc.tile_pool(name="ysp", bufs=NPC))
        yp = ctx.enter_context(tc.tile_pool(name="yp", bufs=10))
        stp = ctx.enter_context(tc.tile_pool(name="stp", bufs=36))
        # PSUM pools (8 banks)
        accp = ctx.enter_context(
            tc.tile_pool(name="accp", bufs=4 if fast else 3, space="PSUM"))
        trp = ctx.enter_context(tc.tile_pool(name="trp", bufs=3, space="PSUM"))
        if not fast:
            gbp = ctx.enter_context(tc.tile_pool(name="gbp", bufs=2, space="PSUM"))

        # ---- weight slab DMA plumbing (SWDGE queue is weights-only; small
        # inputs ride the HWDGE sync queue so slab descriptors are generated
        # from t=0) ----
        def dma_slab_part(n, t, off, end):
            nc.gpsimd.dma_start(out=t[:, off:end], in_=wsd[n][:, off:end])

        def dma_slab(n, split3=False, splitw1=False):
            t = slabp.tile([128, SLAB_LEN], bf16, tag="slab", name="slab")
            if splitw1:
                third = KC1 // 3 * HID
                dma_slab_part(n, t, W1_OFF, W1_OFF + third)
                dma_slab_part(n, t, W1_OFF + third, W1_OFF + 2 * third)
                dma_slab_part(n, t, W1_OFF + 2 * third, W2_OFF)
            else:
                dma_slab_part(n, t, W1_OFF, W2_OFF)
            if split3:
                dma_slab_part(n, t, W2_OFF, W3_OFF)
                dma_slab_part(n, t, W3_OFF, SLAB_LEN)
            else:
                dma_slab_part(n, t, W2_OFF, SLAB_LEN)
            return t

        # ---- constants (first on the SWDGE queue: ~140KB, delays the slab
        # stream by <0.5us but lets x-setup + PE warm-up start at ~1.5us) ----
        eye = cst.tile([32, 32], f32, tag="eye")
        nc.gpsimd.dma_start(out=eye[:], in_=eyed)
        onesb = cst.tile([128, 32], bf16, tag="onesb")
        nc.vector.memset(onesb[:], 1.0)
        magic = cst.tile([B, 1], i32, tag="magic")
        nc.vector.memset(magic[:], QMAGIC)
        bpt = cst.tile([128, 2], f32, tag="bpt")
        nc.gpsimd.dma_start(out=bpt[:], in_=bpd)
        if fast:
            modt = cst.tile([B, NPC], f32, tag="modt")
            nc.gpsimd.dma_start(out=modt[:], in_=modd)
        else:
            bvb = cst.tile([NPC, BVB_LEN], bf16, tag="bvb")
            nc.gpsimd.dma_start(out=bvb[:], in_=bvbd)
            bvg = cst.tile([NPC, BVG_LEN], f32r, tag="bvg")
            nc.gpsimd.dma_start(out=bvg[:], in_=bvgd)
            sel8r = cst.tile([NPC, NPC * B], f32r, tag="sel8r")
            nc.gpsimd.dma_start(out=sel8r[:], in_=sel8d)
            sel8b = cst.tile([NPC, NPC * B], bf16, tag="sel8b")
            nc.vector.tensor_copy(sel8b[:], sel8r[:])

        def w1c(t, k):
            return t[:, W1_OFF + k * HID:W1_OFF + (k + 1) * HID]

        def w2c(t, j):
            return t[:, W2_OFF + j * HID:W2_OFF + (j + 1) * HID]

        def w3c(t, j):
            return t[:, W3_OFF + j * D:W3_OFF + (j + 1) * D]

        # ---- x setup: xT chunks [128, 32] bf16, k = 0..17 ----
        xT = []
        xe = cst.tile([B, D], f32, tag="xe")
        nc.gpsimd.dma_start(out=xe[:], in_=emb)
        wpt = cst.tile([128, 2, D], bf16, tag="wpt")
        nc.gpsimd.dma_start(out=wpt[:], in_=wpd)
        xeT = []
        for k in range(2):
            pt = trp.tile([128, 32], f32, tag="tr")
            nc.tensor.transpose(pt[:], xe[:, k * 128:(k + 1) * 128], eye[:])
            st = cst.tile([128, 32], bf16, tag=f"xeT{k}")
            nc.vector.tensor_copy(st[:], pt[:])
            xeT.append(st)
        for m in range(2):
            pp = trp.tile([128, 32], f32, tag="tr")
            for k in range(2):
                nc.tensor.matmul(pp[:], wpt[:, k, m * 128:(m + 1) * 128], xeT[k][:],
                                 start=(k == 0), stop=(k == 1))
            xt = xtp.tile([128, 32], bf16, tag="xt")
            nc.vector.tensor_scalar_add(xt[:], pp[:], bpt[:, m:m + 1])
            xT.append(xt)

        ht = cst.tile([16, 128], f32, tag="ht")
        nc.gpsimd.dma_start(out=ht[:], in_=histd)

        # weight slabs follow the small inputs on the SWDGE queue
        slabs = {0: dma_slab(0, splitw1=True)}
        if fast:
            slabs[1] = dma_slab(1)
        pt = trp.tile([128, 16], f32, tag="tr")
        nc.tensor.transpose(pt[:], ht[:], eye[0:16, 0:16])
        histT = cst.tile([128, 16], f32, tag="histT")
        nc.vector.tensor_copy(histT[:], pt[:])
        for c in range(16):
            xt = xtp.tile([128, 32], bf16, tag="xt")
            nc.vector.tensor_scalar_mul(xt[:], onesb[:], histT[:, c:c + 1])
            xT.append(xt)

        # PE warm-up with real matmuls (transpose-mode does NOT count as
        # PE-busy for the HAM clock monitor): open the clock gate to 2.4GHz
        # while the first W1 third streams in.  Ping-pong two PSUM tiles so
        # the drains overlap.
        wps0 = trp.tile([B, D], f32, tag="tr")
        wps1 = trp.tile([B, D], f32, tag="tr")
        for w in range(N_WARMUP):
            nc.tensor.matmul(wps0[:] if w % 2 == 0 else wps1[:],
                             xeT[0][:], wpt[:, 0, :], start=True, stop=True)

        # ---- pipeline pieces ----
        ycs = {}
        stats = {}

        def g1_alloc():
            return accp.tile([B, HID], f32, tag="acc", name="p1")

        def g1_group(n, t, p1, k0, k1):
            for k in range(k0, k1):
                nc.tensor.matmul(p1[:], xT[k][:], w1c(t, k),
                                 start=(fast and k == 0), stop=(k == KC1 - 1))

        def g1_bias(n, p1):
            nc.tensor.matmul(p1[:], sel8b[:, n * B:(n + 1) * B],
                             bvb[:, B1_OFF:B1_OFF + HID], start=True, stop=False)

        def gelu_chunks(p, h):
            for j in range(KC2):
                nc.scalar.activation(h[:, j * 128:(j + 1) * 128],
                                     p[:, j * 128:(j + 1) * 128], GELU)

        def transpose4(h):
            hT = []
            for j in range(KC2):
                pt = trp.tile([128, 32], f32, tag="tr")
                nc.tensor.transpose(pt[:], h[:, j * 128:(j + 1) * 128], eye[:])
                st = htp.tile([128, 32], bf16, tag="hT")
                nc.vector.tensor_copy(st[:], pt[:])
                hT.append(st)
            return hT

        def warm_mms(k):
            # keep the HAM clock gate open through phases with no real
            # matmul work (the last neuron's epilogue)
            for w in range(k):
                nc.tensor.matmul(wps0[:] if w % 2 == 0 else wps1[:],
                                 xeT[0][:], wpt[:, 0, :], start=True, stop=True)

        def gemm2_mms(n, t, h1T):
            p2 = accp.tile([B, HID], f32, tag="acc")
            if not fast:
                nc.tensor.matmul(p2[:], sel8b[:, n * B:(n + 1) * B],
                                 bvb[:, B2_OFF:B2_OFF + HID], start=True, stop=False)
            for j in range(KC2):
                nc.tensor.matmul(p2[:], h1T[j][:], w2c(t, j),
                                 start=(fast and j == 0), stop=(j == KC2 - 1))
            return p2

        def w2b(t, k, m):
            # lhsT chunk for weights-stationary GEMM2: [hid1 128k..+128 (K),
            # hid2 128m..+128 (M)] — slab layout already matches
            off = W2_OFF + k * HID + m * 128
            return t[:, off:off + 128]

        def gemm2_b(n, t, h1T):
            # weights-stationary GEMM2: psum2T [128, 4, 32] comes out with
            # hid2 on partitions, so gelu2 directly produces the transposed
            # h2T chunks GEMM3 needs (no layer-2 PE transposes / DVE casts)
            p2t = accp.tile([128, KC2, B], f32, tag="acc", name="p2t")
            for m in range(KC2):
                for k in range(KC2):
                    nc.tensor.matmul(p2t[:, m, :], w2b(t, k, m), h1T[k][:],
                                     start=(k == 0), stop=(k == KC2 - 1))
            h2T = []
            for m in range(KC2):
                st = htp.tile([128, 32], bf16, tag="hT", name="h2T")
                nc.scalar.activation(st[:], p2t[:, m, :], GELU)
                h2T.append(st)
            return h2T

        def gemm3_stats(n, t, h2T):
            p3 = accp.tile([B, D], f32, tag="acc")
            if not fast:
                nc.tensor.matmul(p3[:], sel8b[:, n * B:(n + 1) * B],
                                 bvb[:, B3_OFF:B3_OFF + D], start=True, stop=False)
            for j in range(KC2):
                nc.tensor.matmul(p3[:], h2T[j][:], w3c(t, j),
                                 start=(fast and j == 0), stop=(j == KC2 - 1))
            # LN stats: rs = sum(y) on DVE; yc = y - rs/D; ssq = sum(yc^2)
            # on ACT (Square shares Gelu's table set: no table reload)
            rs = stp.tile([B, 1], f32, tag="st")
            nc.vector.tensor_reduce(rs[:], p3[:], mybir.AxisListType.X, ADD)
            nmu = stp.tile([B, 1], f32, tag="st")
            nc.vector.tensor_scalar_mul(nmu[:], rs[:], -1.0 / D)
            yc = ysp.tile([B, D], f32, tag="ys")
            nc.vector.tensor_scalar_add(yc[:], p3[:], nmu[:])
            sqs = yp.tile([B, D], f32, tag="y")
            ssq = stp.tile([B, 1], f32, tag="st")
            nc.scalar.activation(sqs[:], yc[:], SQUARE, accum_out=ssq[:])
            ycs[n] = yc
            stats[n] = ssq

        def rsqrt_dve(ssq):
            # inv = 1/sqrt(ssq/D + eps): Quake seed + two Newton steps (DVE)
            msq = stp.tile([B, 1], f32, tag="st")
            nc.vector.tensor_scalar(msq[:], ssq[:], 1.0 / D, LN_EPS, MULT, ADD)
            shi = stp.tile([B, 1], i32, tag="sti")
            nc.vector.tensor_scalar(shi[:], msq[:].bitcast(i32), 1, None, ASR)
            x0i = stp.tile([B, 1], i32, tag="sti")
            nc.vector.tensor_sub(x0i[:], magic[:], shi[:])
            x0 = x0i[:].bitcast(f32)
            u = stp.tile([B, 1], f32, tag="st")
            nc.vector.scalar_tensor_tensor(u[:], x0, x0, msq[:], MULT, MULT)
            v = stp.tile([B, 1], f32, tag="st")
            nc.vector.tensor_scalar(v[:], u[:], -0.5, 1.5, MULT, ADD)
            x1 = stp.tile([B, 1], f32, tag="st")
            nc.vector.tensor_mul(x1[:], v[:], x0)
            u2 = stp.tile([B, 1], f32, tag="st")
            nc.vector.scalar_tensor_tensor(u2[:], x1[:], x1[:], msq[:], MULT, MULT)
            v2 = stp.tile([B, 1], f32, tag="st")
            nc.vector.tensor_scalar(v2[:], u2[:], -0.5, 1.5, MULT, ADD)
            inv = stp.tile([B, 1], f32, tag="st")
            nc.vector.tensor_mul(inv[:], v2[:], x1[:])
            return inv

        def emit_B(n):
            yc, ssq = ycs[n], stats[n]
            inv = rsqrt_dve(ssq)
            if fast:
                invm = stp.tile([B, 1], f32, tag="st")
                nc.vector.tensor_mul(invm[:], inv[:], modt[:, n:n + 1])
                yo = yp.tile([B, D], f32, tag="y")
                nc.vector.tensor_scalar_mul(yo[:], yc[:], invm[:])
            else:
                gb = gbp.tile([B, 2 * D], f32, tag="gb")
                nc.tensor.matmul(gb[:], sel8r[:, n * B:(n + 1) * B], bvg[:],
                                 start=True, stop=True)
                yg = yp.tile([B, D], f32, tag="y")
                nc.vector.scalar_tensor_tensor(
                    yg[:], yc[:], inv[:], gb[:, 0:D], MULT, MULT)
                yo = yp.tile([B, D], f32, tag="y")
                nc.vector.tensor_add(yo[:], yg[:], gb[:, D:2 * D])
            nc.sync.dma_start(out=out[:, n, :], in_=yo[:])

        if fast:
            # software-pipelined: GEMM1 of neuron n+1 interleaved (in 6-MM
            # groups) with neuron n's gelu/transpose/GEMM2/GEMM3 phases so
            # the PE always has real matmul work in every HAM window.
            p1s = {0: g1_alloc()}
            g1_group(0, slabs[0], p1s[0], 0, KC1)
            for n in range(NPC):
                nxt = n + 1
                if nxt < NPC:
                    if nxt not in slabs:
                        slabs[nxt] = dma_slab(nxt, split3=(nxt == NPC - 1))
                    p1s[nxt] = g1_alloc()
                h1 = hp.tile([B, HID], f32, tag="h")
                gelu_chunks(p1s[n], h1)
                if nxt < NPC:
                    g1_group(nxt, slabs[nxt], p1s[nxt], 0, 6)
                else:
                    warm_mms(3)
                h1T = transpose4(h1)
                if nxt < NPC:
                    g1_group(nxt, slabs[nxt], p1s[nxt], 6, 12)
                else:
                    warm_mms(3)
                h2T = gemm2_b(n, slabs[n], h1T)
                if nxt < NPC:
                    g1_group(nxt, slabs[nxt], p1s[nxt], 12, KC1)
                else:
                    warm_mms(3)
                gemm3_stats(n, slabs[n], h2T)
                if n > 0:
                    emit_B(n - 1)
            emit_B(NPC - 1)
        else:
            # general path: plain per-neuron pipeline with B lagging one
            def emit_A(n, split3=False):
                t = slabs[n] if n in slabs else dma_slab(n, split3=split3)
                p1 = g1_alloc()
                g1_bias(n, p1)
                g1_group(n, t, p1, 0, KC1)
                h1 = hp.tile([B, HID], f32, tag="h")
                gelu_chunks(p1, h1)
                h1T = transpose4(h1)
                p2 = gemm2_mms(n, t, h1T)
                h2 = hp.tile([B, HID], f32, tag="h")
                gelu_chunks(p2, h2)
                h2T = transpose4(h2)
                gemm3_stats(n, t, h2T)

            for n in range(NPC):
                emit_A(n, split3=(n == NPC - 1))
                if n > 0:
                    emit_B(n - 1)
            emit_B(NPC - 1)

    nc.compile()
    return nc


def _get_program(fast):
    key = "fast" if fast else "general"
    if key not in _CACHE:
        _CACHE[key] = _build_program(fast)
    return _CACHE[key]


def _prep_in_maps(input_embedding, pre_activations, Wp, bp, W1, b1, W2, b2, W3,
                  b3, gamma, beta, tick):
    emb = np.asarray(input_embedding, dtype=np.float32)
    hist = np.asarray(pre_activations, dtype=np.float32)
    Wp = np.asarray(Wp, dtype=np.float32)
    bp = np.asarray(bp, dtype=np.float32)
    W1 = np.asarray(W1, dtype=np.float32)
    b1 = np.asarray(b1, dtype=np.float32)
    W2 = np.asarray(W2, dtype=np.float32)
    b2 = np.asarray(b2, dtype=np.float32)
    W3 = np.asarray(W3, dtype=np.float32)
    b3 = np.asarray(b3, dtype=np.float32)
    gamma = np.asarray(gamma, dtype=np.float32)
    beta = np.asarray(beta, dtype=np.float32)

    fast = (not b1.any() and not b2.any() and not b3.any() and not beta.any()
            and bool(np.all(gamma == 1.0)))

    i = np.arange(N_NEURONS, dtype=np.float64)
    freq = FMIN * (FMAX / FMIN) ** (i / (N_NEURONS - 1))
    phase = np.mod(i * 2.3571, 2.0 * math.pi)
    t = float(np.asarray(tick)) * TICK_INTERVAL
    mod = (1.0 + 0.5 * np.sin(2.0 * math.pi * freq * t + phase)).astype(np.float32)

    histd = np.ascontiguousarray(hist.reshape(16, 128))
    bpd = np.ascontiguousarray(bp.reshape(2, 128).T)
    eyed = np.eye(32, dtype=np.float32)
    wpd = np.ascontiguousarray(
        Wp.reshape(2, 128, D).transpose(1, 0, 2)).astype(ml_dtypes.bfloat16)

    # weight slab: per (neuron, partition) one contiguous bf16 run
    # [W1 18x512 | W2 4x512 | W3 4x256], partition = contraction row % 128
    W1r = W1.reshape(N_NEURONS, KC1, 128, HID).transpose(0, 2, 1, 3) \
        .reshape(N_NEURONS, 128, KC1 * HID)
    W2r = W2.reshape(N_NEURONS, KC2, 128, HID).transpose(0, 2, 1, 3) \
        .reshape(N_NEURONS, 128, KC2 * HID)
    W3r = W3.reshape(N_NEURONS, KC2, 128, D).transpose(0, 2, 1, 3) \
        .reshape(N_NEURONS, 128, KC2 * D)
    wslab = np.ascontiguousarray(
        np.concatenate([W1r, W2r, W3r], axis=2)).astype(ml_dtypes.bfloat16)

    in_maps = []
    for c in range(N_CORES):
        s = slice(c * NPC, (c + 1) * NPC)
        im = {
            "emb": emb,
            "wpd": wpd,
            "bpd": bpd,
            "histd": histd,
            "eyed": eyed,
            "wsd": wslab[s],
        }
        if fast:
            im["modd"] = np.ascontiguousarray(
                np.tile(mod[s][None, :], (B, 1)).astype(np.float32))
        else:
            gm = (gamma * mod[:, None]).astype(np.float32)
            bm = (beta * mod[:, None]).astype(np.float32)
            sel8 = np.zeros((NPC, NPC * B), dtype=np.float32)
            for n in range(NPC):
                sel8[n, n * B:(n + 1) * B] = 1.0
            im["bvbd"] = np.ascontiguousarray(
                np.concatenate([b1[s], b2[s], b3[s]], axis=1)
            ).astype(ml_dtypes.bfloat16)
            im["bvgd"] = np.ascontiguousarray(
                np.concatenate([gm[s], bm[s]], axis=1))
            im["sel8d"] = sel8
        in_maps.append(im)
    return fast, in_maps


def run(inputs, trace=False):
    fast, in_maps = _prep_in_maps(**inputs)
    nc = _get_program(fast)
    br = run_bass_kernel_spmd(nc, in_maps, core_ids=list(range(N_CORES)),
                              trace=trace)
    out = np.concatenate([r["out"] for r in br.results], axis=1)
    return np.ascontiguousarray(out, dtype=np.float32), br


def kernel(**inputs) -> np.ndarray:
    out, _ = run(inputs, trace=False)
    return out
